# revision 20
# baseline (speedup 1.0000x reference)
"""Trainium2 Bass kernel for nn_ExpertGather (MoE gather + per-expert GEMM).

Reference computation (B=8, T=8192, I=512, E=16, K=1024, J=512):
    gathered[b,e,k,:] = x[b, Ind[b,e,k], :]
    out[b,e,k,:]      = gathered[b,e,k,:] @ W[e]

Sharding: expert-parallel across 8 NeuronCores. Core c owns experts
[2c, 2c+1]; x is replicated, Ind/W/out are sharded on E.

Per (b, e_local) pair on a core:
  1. SWDGE dma_gather(transpose=True): 1024 token rows of x[b] (fp16,
     1 KiB each) from HBM -> SBUF tile GT[128, 4, 1024] already in
     lhsT layout: GT[p, c, n] = x[b, idx_n, c*128+p]. No PE transpose
     or DVE copy needed.
  2. Per 128-token tile: 4 accumulating matmuls (contraction I=512 in
     128-chunks), lhsT = GT[:, ic, tile], rhs = W[e] chunk [128, 512]
     -> fp32 PSUM [128tok, 512].
  3. ACT copy PSUM->SBUF (fp32->fp16), HWDGE store 1 MiB per pair.

PRECISION: x/W host-cast to fp16; products accumulate in fp32 PSUM;
output stored fp16 (host upcasts). Measured end-to-end rel err ~5e-4
vs the fp32 reference (gate is 2e-2).
"""

import sys

import numpy as np

if "/opt/trn_rl_repo" not in sys.path:
    sys.path.insert(0, "/opt/trn_rl_repo")

B, T, I = 8, 8192, 512
E, K, J = 16, 1024, 512
NCORES = 8
E_LOCAL = E // NCORES  # 2 experts per core
PAIRS = B * E_LOCAL  # 16 (b, e_local) pairs per core
KT = K // 128  # 8 token tiles per pair
IC = I // 128  # 4 contraction chunks
IDX_W = K // 16  # 64 idxs per partition row (16-partition wrap)

_CACHE: dict = {}


def _build_nc(repeat=1):
    """Build the Bass module. `repeat` re-emits the whole computation that
    many times inside one NEFF (timing use only: slope between repeat counts
    cancels per-call dispatch overhead)."""
    import concourse.mybir as mybir
    import concourse.tile as tile
    from concourse import bacc

    f32 = mybir.dt.float32
    f16 = mybir.dt.float16
    i16 = mybir.dt.int16

    nc = bacc.Bacc("TRN2", target_bir_lowering=False, debug=False)
    # x rows 0..B*T-1: activations; rows B*T..: W[e] rows for e>=1, so the
    # late weight loads can ride the SWDGE gather path (dependency-anchored
    # on the second idx chunk -> they can't preempt head-critical gathers).
    XROWS = B * T + (E_LOCAL - 1) * I
    x = nc.dram_tensor("x", [XROWS, I], f16, kind="ExternalInput")
    w = nc.dram_tensor("w", [128, IC, J], f16, kind="ExternalInput")  # e=0 only
    # idx slot PAIRS+k (k=0..E_LOCAL-2): iota(512) wrap for the W[k+1] gather
    idx = nc.dram_tensor(
        "idx", [128, PAIRS + E_LOCAL - 1, IDX_W], i16, kind="ExternalInput"
    )
    out = nc.dram_tensor("out", [B, E_LOCAL, K, J], f16, kind="ExternalOutput")

    WARM_MM = 52  # narrow dummy matmuls that hold PE busy (p-state ramp)
    #   until the first real lhsT chunk lands; each is ~107 ns at mid p-state
    K2 = K // 2  # 512-token half-pair gather granularity
    TH = KT // 2  # 4 token tiles per half
    K4 = K // 4  # 256-token first-pair gather granularity (shorter head)

    with tile.TileContext(nc) as tc:
        with (
            tc.tile_pool(name="const", bufs=1) as const_pool,
            tc.tile_pool(name="gt", bufs=8) as gt_pool,
            tc.tile_pool(name="osb", bufs=6) as o_pool,
            tc.tile_pool(name="warmps", bufs=1, space="PSUM") as warm_pool,
            tc.tile_pool(name="ops", bufs=7, space="PSUM") as ops_pool,
        ):
            # PE warm-up stream: matmuls over an SBUF-resident zero tile.
            # Results are discarded; this only keeps the PE array busy from
            # t~0 so the p-state ramp completes before real matmuls start.
            warm = const_pool.tile([128, 128], f16)
            nc.gpsimd.memset(warm[:], 0.0)
            warm_ps = warm_pool.tile([128, 128], f32)
            for wi in range(WARM_MM):
                nc.tensor.matmul(
                    warm_ps[:],
                    warm[:],
                    warm[:],
                    start=(wi == 0),
                    stop=(wi == WARM_MM - 1),
                )

            # pair-0 idx slice first: it gates the whole pipeline head
            idx_sb = const_pool.tile([128, PAIRS + E_LOCAL - 1, IDX_W], i16)
            nc.sync.dma_start(idx_sb[:, 0:1], idx[:, 0:1])
            nc.sync.dma_start(idx_sb[:, 1:], idx[:, 1:])
            w_sb = const_pool.tile([128, E_LOCAL, IC, J], f16)
            # Pairs run expert-major ((b,e=0) x8 then e=1 x8), so only w[e0]
            # is needed early; it loads on the sync queue up front. w[e>0]
            # arrives via iota-index SWDGE gathers from x's tail rows.
            nc.sync.dma_start(w_sb[:, 0], w[:])

            for q in range(PAIRS * repeat):
                qp = q % PAIRS
                b, e = qp % B, qp // B
                for half in range(2):
                    if q == 1 and half == 1:
                        for e1 in range(1, E_LOCAL):
                            # non-transpose gather of 512 iota-indexed rows:
                            # w_sb[p, ic, :] = x_tail_row[ic*128+p]
                            nc.gpsimd.dma_gather(
                                w_sb[:, e1],
                                x[B * T + (e1 - 1) * I : B * T + e1 * I],
                                idx_sb[:, PAIRS + e1 - 1, 0 : I // 16],
                                I,
                                I,
                                J,
                            )
                    # Transposed half-gather:
                    #   gt[p, ic, n] = x[b*T + idx[half*K2+n], ic*128+p]
                    if q == 0:
                        # first pair: 2 quarter-gathers per half (short head)
                        gq = []
                        for quar in range(2):
                            g = gt_pool.tile([128, IC, K4], f16)
                            gq.append(g)
                            c0 = (2 * half + quar) * (IDX_W // 4)
                            nc.gpsimd.dma_gather(
                                g[:],
                                x[b * T : (b + 1) * T],
                                idx_sb[:, 0, c0 : c0 + IDX_W // 4],
                                K4,
                                K4,
                                I,
                                transpose=True,
                            )
                        lhs = lambda th, ic, gq=gq: gq[th // 2][
                            :, ic, (th % 2) * 128 : (th % 2) * 128 + 128
                        ]
                    else:
                        gt = gt_pool.tile([128, IC, K2], f16)
                        nc.gpsimd.dma_gather(
                            gt[:],
                            x[b * T : (b + 1) * T],
                            idx_sb[
                                :,
                                b * E_LOCAL + e,  # host pair index
                                half * (IDX_W // 2) : (half + 1) * (IDX_W // 2),
                            ],
                            K2,
                            K2,
                            I,
                            transpose=True,
                        )
                        lhs = lambda th, ic, gt=gt: gt[
                            :, ic, th * 128 : (th + 1) * 128
                        ]
                    last_pair = qp == PAIRS - 1
                    o_sb = None if last_pair else o_pool.tile([128, TH, J], f16)
                    for th in range(TH):
                        o_ps = ops_pool.tile([128, J], f32)
                        for ic in range(IC):
                            nc.tensor.matmul(
                                o_ps[:],
                                lhs(th, ic),
                                w_sb[:, e, ic, :],
                                start=(ic == 0),
                                stop=(ic == IC - 1),
                            )
                        if last_pair:
                            # per-tile store: shortest possible drain tail
                            t0 = (half * TH + th) * 128
                            o_t = o_pool.tile([128, 1, J], f16)
                            if half == 1 and th == TH - 1:
                                # final tile: DVE copy is ~220ns quicker and
                                # its queue is idle (no decode backlog)
                                nc.vector.tensor_copy(out=o_t[:, 0, :], in_=o_ps[:])
                            else:
                                nc.scalar.copy(out=o_t[:, 0, :], in_=o_ps[:])
                            nc.sync.dma_start(
                                out[b, e, t0 : t0 + 128].rearrange(
                                    "(blk p) j -> p blk j", p=128
                                ),
                                o_t[:],
                            )
                        else:
                            nc.scalar.copy(out=o_sb[:, th, :], in_=o_ps[:])
                    if not last_pair:
                        nc.sync.dma_start(
                            out[b, e, half * K2 : (half + 1) * K2].rearrange(
                                "(blk p) j -> p blk j", p=128
                            ),
                            o_sb[:],
                        )
    nc.compile()
    return nc


def _get_nc(repeat=1):
    key = ("nc", repeat)
    if key not in _CACHE:
        _CACHE[key] = _build_nc(repeat)
    return _CACHE[key]


def _wrap16(vals):
    """idx wrap layout: unwrapped[j] = idxs[j % 16, j // 16], tiled to 128."""
    wrapped = vals.astype(np.int16).reshape(-1, 16).T  # [16, n//16]
    return np.tile(wrapped, (8, 1))  # [128, n//16]


def _make_in_maps(x, Ind, W):
    x16 = np.asarray(x, dtype=np.float32).astype(np.float16).reshape(B * T, I)
    Ind = np.asarray(Ind)
    W = np.asarray(W, dtype=np.float32)
    NSLOT = PAIRS + E_LOCAL - 1
    in_maps = []
    for c in range(NCORES):
        wl = W[c * E_LOCAL : (c + 1) * E_LOCAL].astype(np.float16)  # [E_LOCAL, I, J]
        # sync-loaded w[e=0]: w_host[p, ic, j] = wl[0, ic*128 + p, j]
        w_host = np.ascontiguousarray(
            wl[0].reshape(IC, 128, J).transpose(1, 0, 2)
        )
        # x tail rows: W[e] rows for e >= 1 (gather-loaded on device)
        x_dev = np.ascontiguousarray(
            np.concatenate([x16, wl[1:].reshape((E_LOCAL - 1) * I, J)], axis=0)
        )
        idxs = np.zeros((128, NSLOT, IDX_W), np.int16)
        for b in range(B):
            for e in range(E_LOCAL):
                q = b * E_LOCAL + e
                idxs[:, q, :] = _wrap16(Ind[b, c * E_LOCAL + e])
        for e1 in range(1, E_LOCAL):
            idxs[:, PAIRS + e1 - 1, 0 : I // 16] = _wrap16(np.arange(I))
        in_maps.append({"x": x_dev, "w": w_host, "idx": idxs})
    return in_maps


def run(x, Ind, W, trace=False):
    """Run the kernel; returns (out, BassKernelResults)."""
    import os

    from concourse.bass_utils import run_bass_kernel_spmd

    nc = _get_nc()
    in_maps = _make_in_maps(x, Ind, W)
    try:
        res = run_bass_kernel_spmd(
            nc, in_maps, core_ids=list(range(NCORES)), trace=trace
        )
    except ModuleNotFoundError:
        # axon NTFF profiling hook absent (no antenv.axon_hooks) — retry
        # with tracing force-disabled.
        os.environ["BASS_NEVER_TRACE"] = "1"
        res = run_bass_kernel_spmd(
            nc, in_maps, core_ids=list(range(NCORES)), trace=False
        )
    outs = [r["out"] for r in res.results]  # each [B, E_LOCAL, K, J]
    full = np.concatenate(outs, axis=1)  # experts in core order -> [B, E, K, J]
    return np.ascontiguousarray(full.astype(np.float32)), res


def kernel(x, Ind, W):
    out, _ = run(x, Ind, W, trace=False)
    return out
